# revision 24
# baseline (speedup 1.0000x reference)
"""Causal self-attention (B=2, T=2048, D=1024, H=16, HD=64) on 8 trn2 NeuronCores.

Sharding: core = b*4 + g  (b = batch 0..1, g = head-group 0..3, 4 heads each).
Megatron-style: column-split w_qkv per head group, row-split w_o; the w_o
all-reduce over each batch's 4 cores is done host-side (partial sums).

v2 design (vs. fp32r baseline at ~315us):
  - all matmul operands bf16 (halves DMA, SBUF; same 1 cycle/row on PE)
  - heads packed 2-per-128-partitions; scores matmuls contract K=64 at
    partition bases {0,64} via tile_position -> no zero padding, and q/k
    projection evictions write SBUF directly (no shift DMAs)
  - causal mask applied as 0/1 bf16 multiply on DVE after exp (off PE)
  - softmax denominator via ones-column in v (row 64 of yacc PSUM);
    normalization = tensor_tensor divide on DVE fed by a gpsimd
    partition_broadcast (no 3.3us DVE reciprocals)
  - software-pipelined PE stream: scores(j)/exp(j) lookahead-1 ahead of
    yacc(j-1); next-chunk projections and prev-chunk out-projections are
    injected between attention matmuls so PE never drains (keeps p-state
    at 2.4 GHz)
  - DMAs split across sync (x loads) and gpsimd (weights, denom moves,
    yT odd-head shifts, out stores) queues
"""

import os
from collections import deque

import numpy as np

B, T, D = 2, 2048, 1024
H, HD = 16, 64
LH = 4            # local heads per core
KO = 8            # contraction tiles of 128 over D
NTQ, TQC = 4, 512  # tq chunks
DVE_ = 4 * 65      # v cols incl. ones col per head

_PROG = {}
LAST_RESULT = None


def _build_program(debug_dumps=False):
    import concourse.bass as bass
    from concourse import bacc
    import concourse.tile as tile
    import concourse.mybir as mybir

    f32 = mybir.dt.float32
    bf16 = mybir.dt.bfloat16
    AF = mybir.ActivationFunctionType
    ALU = mybir.AluOpType
    ts = bass.ts

    nc = bacc.Bacc(None, target_bir_lowering=False, debug=True)
    xT_d = nc.dram_tensor("xT", [128, KO, T], bf16, kind="ExternalInput")
    wqk_d = nc.dram_tensor("w_qk", [128, KO, 4, 128], bf16, kind="ExternalInput")
    bqk_d = nc.dram_tensor("b_qk", [128, 4], f32, kind="ExternalInput")
    wv_d = nc.dram_tensor("w_v", [128, KO, DVE_], bf16, kind="ExternalInput")
    bv_d = nc.dram_tensor("b_v", [128, LH, 65], f32, kind="ExternalInput")
    wo_d = nc.dram_tensor("w_o", [128, 2, D], bf16, kind="ExternalInput")
    msk_d = nc.dram_tensor("masks", [128, 4, TQC], bf16, kind="ExternalInput")
    out_d = nc.dram_tensor("out_part", [T, D], f32, kind="ExternalOutput")
    dbg = {}
    if debug_dumps:
        dbg["qk"] = nc.dram_tensor("dbg_qk", [128, 4, T], bf16, kind="ExternalOutput")
        dbg["v"] = nc.dram_tensor("dbg_v", [128, 4 * NTQ, LH, 65], bf16, kind="ExternalOutput")
        dbg["yT"] = nc.dram_tensor("dbg_yT", [128, 2, T], bf16, kind="ExternalOutput")
        dbg["et"] = nc.dram_tensor("dbg_et", [128, 2, TQC], bf16, kind="ExternalOutput")
        dbg["etm"] = nc.dram_tensor("dbg_etm", [128, 2, TQC], bf16, kind="ExternalOutput")
        dbg["d0"] = nc.dram_tensor("dbg_d0", [16, TQC], f32, kind="ExternalOutput")
        dbg["d0b"] = nc.dram_tensor("dbg_d0b", [16, TQC], f32, kind="ExternalOutput")
        dbg["d0c"] = nc.dram_tensor("dbg_d0c", [16, TQC], f32, kind="ExternalOutput")

    with tile.TileContext(nc) as tc:
        with (
            tc.tile_pool(name="big", bufs=1) as big,
            tc.tile_pool(name="xtp", bufs=2) as xtp,
            tc.tile_pool(name="etp", bufs=3) as etp,
            tc.tile_pool(name="dnp", bufs=2) as dnp,
            tc.tile_pool(name="outp", bufs=3) as outp,
            tc.tile_pool(name="ps_s", bufs=2, space="PSUM") as ps_s,
            tc.tile_pool(name="ps_y", bufs=2, space="PSUM") as ps_y,
            tc.tile_pool(name="ps_w", bufs=2, space="PSUM") as ps_w,
        ):
            wqk = big.tile([128, KO, 4, 128], bf16, name="wqk_sb")
            wv = big.tile([128, KO, DVE_], bf16, name="wv_sb")
            wo = big.tile([128, 2, D], bf16, name="wo_sb")
            bqk = big.tile([128, 4], f32, name="bqk_sb")
            bv = big.tile([128, LH, 65], f32, name="bv_sb")
            msk = big.tile([128, 4, TQC], bf16, name="msk_sb")
            # qk: subtile 0,1 = q head pairs (0,1),(2,3); 2,3 = k pairs.
            # within a subtile: even head on partitions 0:64, odd on 64:128
            qk = big.tile([128, 4, T], bf16, name="qk_sb")
            vsb = big.tile([128, 4 * NTQ, LH, 65], bf16, name="v_sb")
            yT = big.tile([128, 2, T], bf16, name="yT_sb")

            # ---- startup DMAs in first-need order, split across 4 queues ----
            xcs = {}
            xcs[0] = xtp.tile([128, KO, TQC], bf16, name="xc_0", tag="xc")
            for lo, hi in ((0, 1), (1, 4), (4, 8)):
                nc.sync.dma_start(wqk[:, lo:hi], wqk_d[:, lo:hi])
                nc.scalar.dma_start(
                    xcs[0][:, lo:hi, :], xT_d[:, lo:hi, 0:TQC]
                )
            nc.gpsimd.dma_start(bqk[:], bqk_d[:])
            nc.gpsimd.dma_start(wv[:], wv_d[:])
            nc.gpsimd.dma_start(bv[:], bv_d[:])
            nc.gpsimd.dma_start(msk[:], msk_d[:])
            xcs[1] = xtp.tile([128, KO, TQC], bf16, name="xc_1", tag="xc")
            nc.gpsimd.dma_start(xcs[1][:], xT_d[:, :, TQC : 2 * TQC])
            nc.gpsimd.dma_start(wo[:], wo_d[:])

            def emit_qk_group(c, s):
                xc = xcs[c]
                pst = ps_w.tile([128, TQC], f32, name=f"pqk_{c}_{s}", tag="work")
                for ko in range(KO):
                    nc.tensor.matmul(
                        pst[:],
                        wqk[:, ko, s, :],
                        xc[:, ko, :],
                        start=(ko == 0),
                        stop=(ko == KO - 1),
                    )
                nc.vector.tensor_scalar_add(
                    qk[:, s, ts(c, TQC)], pst[:], bqk[:, s : s + 1]
                )

            def emit_v_group(c, tbl):
                xc = xcs[c]
                tb = 4 * c + tbl
                pst = ps_w.tile([128, DVE_], f32, name=f"pv_{tb}", tag="work")
                for ko in range(KO):
                    nc.tensor.matmul(
                        pst[:],
                        xc[:, ko, ts(tbl, 128)],
                        wv[:, ko, :],
                        start=(ko == 0),
                        stop=(ko == KO - 1),
                    )
                nc.vector.tensor_add(
                    vsb[:, tb, :, :],
                    pst[:].rearrange("p (h e) -> p h e", h=LH),
                    bv[:],
                )

            def emit_out_group(c, m, n):
                pst = ps_w.tile([128, TQC], f32, name=f"po_{c}_{m}_{n}", tag="work")
                for kt in range(2):
                    nc.tensor.matmul(
                        pst[:],
                        yT[:, kt, ts(4 * c + m, 128)],
                        wo[:, kt, ts(n, TQC)],
                        start=(kt == 0),
                        stop=(kt == 1),
                    )
                ot = outp.tile([128, TQC], f32, name=f"ot_{c}_{m}_{n}", tag="ot")
                if (2 * m + n) % 2 == 0:
                    nc.vector.tensor_copy(ot[:], pst[:])
                    nc.gpsimd.dma_start(out_d[ts(4 * c + m, 128), ts(n, TQC)], ot[:])
                else:
                    nc.scalar.copy(ot[:], pst[:])
                    nc.sync.dma_start(out_d[ts(4 * c + m, 128), ts(n, TQC)], ot[:])

            bg = deque()

            def pump(k):
                for _ in range(min(k, len(bg))):
                    bg.popleft()()

            # ---- chunk 0 projections inline ----
            for s in range(4):
                emit_qk_group(0, s)
            for tbl in range(4):
                emit_v_group(0, tbl)

            for c in range(NTQ):
                cs = ts(c, TQC)
                nb = 4 * (c + 1)
                # prefetch x for chunk c+2
                if c + 2 < NTQ:
                    xcs[c + 2] = xtp.tile(
                        [128, KO, TQC], bf16, name=f"xc_{c+2}", tag="xc"
                    )
                    nc.sync.dma_start(
                        xcs[c + 2][:], xT_d[:, :, ts(c + 2, TQC)]
                    )
                # background PE work for this chunk's attention phase
                if c + 1 < NTQ:
                    for s in range(4):
                        bg.append(lambda c=c + 1, s=s: emit_qk_group(c, s))
                    for tbl in range(4):
                        bg.append(lambda c=c + 1, tbl=tbl: emit_v_group(c, tbl))
                if c >= 1:
                    for m in range(4):
                        for n in range(2):
                            bg.append(
                                lambda c=c - 1, m=m, n=n: emit_out_group(c, m, n)
                            )
                n_j = 2 * nb

                for p in range(2):  # head pair
                    psy = {}
                    for e in range(2):
                        psy[e] = ps_y.tile(
                            [128, TQC], f32, name=f"psy_{c}_{p}_{e}", tag="psy"
                        )
                    prev = None
                    for j in range(nb):
                        # inject background PE work first so the in-order PE
                        # queue has ready matmuls ahead of dependent ones
                        rem_j = (1 - p) * nb + (nb - j)
                        k = -(-len(bg) // max(rem_j, 1))
                        if p == 0 and j == 0:
                            k = max(k, 3)  # cover the chunk-boundary stall
                        pump(k)
                        # scores for both heads of the pair (K=64 quadrants)
                        pss = ps_s.tile(
                            [128, 2, TQC], f32, name=f"pss_{c}_{p}_{j}", tag="pss"
                        )
                        for e in range(2):
                            pb = 64 * e
                            nc.tensor.matmul(
                                pss[:, e, :],
                                qk[pb : pb + 64, 2 + p, ts(j, 128)],
                                qk[pb : pb + 64, p, cs],
                                start=True,
                                stop=True,
                            )
                        et = etp.tile(
                            [128, 2, TQC], bf16, name=f"et_{c}_{p}_{j}", tag="et"
                        )
                        nc.scalar.activation(et[:], pss[:], AF.Exp, scale=1.0 / 32.0)
                        if j >= 4 * c:  # diagonal block: 0/1 causal mask
                            v_ = j - 4 * c
                            etm = etp.tile(
                                [128, 2, TQC], bf16, name=f"etm_{c}_{p}_{j}", tag="et"
                            )
                            for e in range(2):
                                nc.vector.tensor_mul(
                                    etm[:, e, :], et[:, e, :], msk[:, v_, :]
                                )
                            if debug_dumps and c == 0 and p == 0 and j == 0:
                                nc.sync.dma_start(dbg["et"][:], et[:])
                                nc.sync.dma_start(dbg["etm"][:], etm[:])
                            et = etm
                        if prev is not None:
                            pj, pet = prev
                            for e in range(2):
                                nc.tensor.matmul(
                                    psy[e][0:65, :],
                                    vsb[:, pj, 2 * p + e, :],
                                    pet[:, e, :],
                                    start=(pj == 0),
                                    stop=False,
                                )
                        prev = (j, et)
                    pj, pet = prev
                    for e in range(2):
                        nc.tensor.matmul(
                            psy[e][0:65, :],
                            vsb[:, pj, 2 * p + e, :],
                            pet[:, e, :],
                            start=(pj == 0),
                            stop=True,
                        )
                    # evict yu+denominator to SBUF at once, freeing the PSUM
                    # bank immediately (next pair's yacc reuses it); then the
                    # 1/denom chain (gpsimd normalize_recip: no ACT table
                    # switching, and reciprocal_approx_fast/DVE reciprocal are
                    # broken/slow on this hw) runs off the critical path
                    last = c == NTQ - 1 and p == 1
                    for e in range(2):
                        d0 = dnp.tile([1, TQC], f32, name=f"d0_{c}_{p}_{e}", tag="d0")
                        if last:
                            # tail: no exps follow, so one ACT table switch is
                            # free -> short ACT Reciprocal chain (1.2e-5 rel,
                            # verified on hw), psum read directly
                            yus = psy[e]
                            dh = dnp.tile(
                                [128, TQC], f32, name=f"dhl_{e}", tag="dhl"
                            )
                            eng = nc.scalar
                            eng.add_instruction(
                                mybir.InstActivation(
                                    name=nc.get_next_instruction_name(),
                                    func=AF.Reciprocal,
                                    ins=[
                                        eng.lower_ap(psy[e][64:65, :]),
                                        mybir.ImmediateValue(
                                            dtype=f32, value=0.0
                                        ),
                                        mybir.ImmediateValue(
                                            dtype=f32, value=1.0
                                        ),
                                        mybir.ImmediateValue(
                                            dtype=f32, value=0.0
                                        ),
                                    ],
                                    outs=[eng.lower_ap(dh[64:65, :])],
                                )
                            )
                            nc.sync.dma_start(d0[:], dh[64:65, :])
                        else:
                            yus = dnp.tile(
                                [65, TQC], f32, name=f"yu_{c}_{p}_{e}", tag="yu"
                            )
                            nc.vector.tensor_copy(yus[:], psy[e][0:65, :])
                            dcol = dnp.tile(
                                [128, 4], f32, name=f"dc_{c}_{p}_{e}", tag="dc"
                            )
                            nc.sync.dma_start(dcol[:], yus[64:65, :])
                            junk = dnp.tile(
                                [128, 4], f32, name=f"jk_{c}_{p}_{e}", tag="jk"
                            )
                            for i in range(4):
                                nc.gpsimd.normalize_recip(
                                    junk[:, i : i + 1],
                                    dcol[:, i : i + 1],
                                    dcol[:, i : i + 1],
                                )
                            nc.sync.dma_start(d0[:], dcol[:])
                        if debug_dumps:
                            idx = 4 * c + 2 * p + e
                            nc.sync.dma_start(dbg["d0"][idx : idx + 1, :], d0[:])
                        rb = dnp.tile([64, TQC], f32, name=f"rb_{c}_{p}_{e}", tag="rb")
                        nc.gpsimd.partition_broadcast(rb[:], d0[:])
                        if e == 0:
                            nc.vector.tensor_mul(
                                yT[0:64, p, cs], yus[0:64, :], rb[:]
                            )
                        else:
                            tmp = dnp.tile(
                                [64, TQC], bf16, name=f"tmp_{c}_{p}", tag="tmpy"
                            )
                            nc.vector.tensor_mul(tmp[:], yus[0:64, :], rb[:])
                            nc.sync.dma_start(yT[64:128, p, cs], tmp[:])

            pump(len(bg))

            # final chunk's out projection
            for m in range(4):
                for n in range(2):
                    emit_out_group(NTQ - 1, m, n)

            if debug_dumps:
                nc.sync.dma_start(dbg["qk"][:], qk[:])
                nc.sync.dma_start(dbg["v"][:], vsb[:])
                nc.sync.dma_start(dbg["yT"][:], yT[:])

    nc.finalize()
    return nc


def _host_inputs(x, w_qkv, b_qkv, w_o, b_o):
    import ml_dtypes

    bf16 = ml_dtypes.bfloat16

    xT = []
    for b in range(B):
        t = np.ascontiguousarray(x[b].T)  # [D, T]
        xT.append(
            np.ascontiguousarray(
                t.reshape(KO, 128, T).swapaxes(0, 1).astype(bf16)
            )
        )

    p = np.arange(128)[:, None]
    f = np.arange(TQC)[None, :]
    masks = np.stack(
        [(f >= p + 128 * v).astype(np.float32) for v in range(4)], axis=1
    )  # [128, 4, 512]
    masks = np.ascontiguousarray(masks.astype(bf16))

    in_maps = []
    for core in range(8):
        b, g = divmod(core, 4)
        # qk groups: s=0,1 -> q head pairs; s=2,3 -> k head pairs
        wqk_g = np.zeros((D, 4, 128), dtype=np.float32)
        bqk_g = np.zeros((128, 4), dtype=np.float32)
        for s in range(4):
            base = 0 if s < 2 else D  # q vs k
            pair = s % 2
            cols = slice(
                base + g * 256 + pair * 128, base + g * 256 + pair * 128 + 128
            )
            wqk_g[:, s, :] = w_qkv[:, cols]
            bqk_g[:, s] = b_qkv[cols]
        wqk_g = np.ascontiguousarray(
            wqk_g.reshape(KO, 128, 4, 128).swapaxes(0, 1).astype(bf16)
        )

        w_v = np.zeros((D, DVE_), dtype=np.float32)
        b_v = np.zeros((LH, 65), dtype=np.float32)
        for h in range(LH):
            vcols = slice(2 * D + g * 256 + h * 64, 2 * D + g * 256 + (h + 1) * 64)
            w_v[:, h * 65 : h * 65 + 64] = w_qkv[:, vcols]
            b_v[h, 0:64] = b_qkv[vcols]
            b_v[h, 64] = 1.0  # ones column (weight col stays 0)
        w_v = np.ascontiguousarray(
            w_v.reshape(KO, 128, DVE_).swapaxes(0, 1).astype(bf16)
        )
        b_v_bc = np.ascontiguousarray(
            np.broadcast_to(b_v[None], (128, LH, 65)).copy()
        )

        w_o_g = w_o[g * 256 : (g + 1) * 256, :]  # [256, D]
        w_o_g = np.ascontiguousarray(
            w_o_g.reshape(2, 128, D).swapaxes(0, 1).astype(bf16)
        )

        in_maps.append(
            {
                "xT": xT[b],
                "w_qk": wqk_g,
                "b_qk": np.ascontiguousarray(bqk_g),
                "w_v": w_v,
                "b_v": b_v_bc,
                "w_o": w_o_g,
                "masks": masks,
            }
        )
    return in_maps


def kernel(x, w_qkv, b_qkv, w_o, b_o):
    global LAST_RESULT
    from concourse.bass_utils import run_bass_kernel_spmd

    x = np.asarray(x, dtype=np.float32)
    w_qkv = np.asarray(w_qkv, dtype=np.float32)
    b_qkv = np.asarray(b_qkv, dtype=np.float32)
    w_o = np.asarray(w_o, dtype=np.float32)
    b_o = np.asarray(b_o, dtype=np.float32)

    if "nc" not in _PROG:
        _PROG["nc"] = _build_program()
    nc = _PROG["nc"]

    in_maps = _host_inputs(x, w_qkv, b_qkv, w_o, b_o)

    trace = bool(os.environ.get("KERNEL_TRACE"))
    res = run_bass_kernel_spmd(nc, in_maps, core_ids=list(range(8)), trace=trace)
    LAST_RESULT = res

    out = np.empty((B, T, D), dtype=np.float32)
    for b in range(B):
        acc = res.results[b * 4]["out_part"].astype(np.float32).copy()
        for g in range(1, 4):
            acc += res.results[b * 4 + g]["out_part"]
        out[b] = acc + b_o[None, :]
    return out
